# revision 9
# baseline (speedup 1.0000x reference)
"""Multi-head attention block (B=4, N=1024, C=1024, H=16, d=64) on 8 TRN2 cores.

Sharding: core = 2*b + hh  (batch b in 0..3, head-half hh in 0..1 -> 8 heads/core).
Each core computes the qkv projection for its 8 heads, attention, and a partial
output projection (its 512 rows of w_proj). Host sums the two partials per
batch and adds b_proj.

Per-core pipeline (all matmul inputs float32r -> 1 PE cycle/row):
  - x^T fed from host, so Y_qk^T[cols,seq] = (W_qk chunks).T @ x^T chunks gives
    q^T,k^T directly; Y_v[seq,vcols] = (x^T chunks).T @ W_v gives V naturally.
    Zero on-chip transposes.
  - per head pair (heads 2p, 2p+1 at partition bases 0/64): S^T[keys,q] =
    k^T.T @ q^T with K=64; the two heads' matmuls target disjoint PE row
    groups and run concurrently. exp on ACT (scale=1/8 folded in), into f32r.
  - AV with V augmented by a ones column: one PSUM accumulation yields both
    att^T[64,q] and the softmax denominators (row 64). Normalize: DVE
    reciprocal (PSUM row), gpsimd partition_broadcast, DVE multiply -> att^T.
  - proj: out[seq,outfeat] = (att^T chunks).T @ w_proj chunks, DVE evict,
    DMA out. QKV production, attention, and eviction pipeline across engines;
    phases interleave per head pair.
"""

import numpy as np

B = 4
N = 1024
C = 1024
H = 16
D = 64
NCORES = 8
SCALE = D ** -0.5


_NC_CACHE = {}


def _build_bass():
    import concourse.mybir as mybir
    from concourse import bacc
    from concourse.tile import TileContext

    dt = mybir.dt
    f32 = dt.float32
    f32r = dt.float32r
    Act = mybir.ActivationFunctionType

    nc = bacc.Bacc(
        "TRN2",
        target_bir_lowering=False,
        debug=False,
        num_devices=NCORES,
        num_swdge_queues=4,
    )

    # ---- DRAM I/O (per-core shards; host prepares layouts) ----
    xT_d = nc.dram_tensor("xT", [C, N], f32, kind="ExternalInput").ap()
    wqk_d = nc.dram_tensor("wqk", [C, 1024], f32, kind="ExternalInput").ap()
    wv_d = nc.dram_tensor("wv", [C, 512], f32, kind="ExternalInput").ap()
    wp_d = nc.dram_tensor("wp", [512, C], f32, kind="ExternalInput").ap()
    bqk_d = nc.dram_tensor("bqk", [128, 8], f32, kind="ExternalInput").ap()
    bv_d = nc.dram_tensor("bv", [128, 512], f32, kind="ExternalInput").ap()
    ones_d = nc.dram_tensor("ones64", [128, 64], f32, kind="ExternalInput").ap()
    y_d = nc.dram_tensor("y", [N, C], f32, kind="ExternalOutput").ap()

    with TileContext(nc) as tc:
        with (
            tc.tile_pool(name="persist", bufs=1) as persist,
            tc.tile_pool(name="yqk_pool", bufs=2) as yqk_pool,
            tc.tile_pool(name="es_pool", bufs=20) as es_pool,
            tc.tile_pool(name="norm", bufs=3) as norm,
            tc.tile_pool(name="psum", bufs=2, space="PSUM") as ps,
            tc.tile_pool(name="psav", bufs=3, space="PSUM") as psav,
        ):
            # persistent SBUF tensors
            vst = persist.tile([128, 8, 8, 65], f32r, tag="vst")  # [keys128, s, h, d+1]
            attr = persist.tile([128, 4, N], f32r, tag="attr")  # att^T normalized
            bqk_t = persist.tile([128, 8], f32, tag="bqk")
            bv_t = persist.tile([128, 512], f32, tag="bv")

            nc.sync.dma_start(bqk_t[:], bqk_d)
            nc.sync.dma_start(bv_t[:], bv_d)

            with tc.tile_pool(name="ph1", bufs=1) as ph1:
                xT = [
                    ph1.tile([128, N], f32r, tag=f"xT{k}", name=f"xT{k}")
                    for k in range(8)
                ]
                wqk = [
                    ph1.tile([128, 1024], f32r, tag=f"wqk{k}", name=f"wqk{k}")
                    for k in range(8)
                ]
                wv = [
                    ph1.tile([128, 512], f32r, tag=f"wv{k}", name=f"wv{k}")
                    for k in range(8)
                ]
                for k in range(8):
                    nc.gpsimd.dma_start(xT[k][:], xT_d[k * 128:(k + 1) * 128, :])
                    nc.gpsimd.dma_start(wv[k][:], wv_d[k * 128:(k + 1) * 128, :])
                    nc.gpsimd.dma_start(wqk[k][:], wqk_d[k * 128:(k + 1) * 128, :])
                # ones column of V-hat
                nc.gpsimd.dma_start(
                    vst[:, :, :, 64],
                    ones_d.rearrange("p (s h) -> p s h", s=8),
                )

                # ---- Y_v [seq, vcols] ----
                for s in range(8):
                    pv = ps.tile([128, 512], f32, tag="s", name=f"pv{s}")
                    for k in range(8):
                        nc.tensor.matmul(
                            pv[:],
                            xT[k][:, s * 128:(s + 1) * 128],
                            wv[k][:],
                            start=(k == 0),
                            stop=(k == 7),
                        )
                    nc.vector.tensor_add(
                        out=vst[:, s, :, 0:64],
                        in0=pv[:].rearrange("p (h d) -> p h d", h=8),
                        in1=bv_t[:].rearrange("p (h d) -> p h d", h=8),
                    )

                # ---- per head-pair pipeline ----
                for p in range(4):
                    # Y_qk^T for this pair's q-cols (chunk p) and k-cols (4+p)
                    ytiles = {}
                    for cc, tagn in ((p, "yq"), (4 + p, "yk")):
                        pq = ps.tile([128, N], f32, tag="s", name=f"pq{cc}")
                        for s in range(2):
                            for k in range(8):
                                nc.tensor.matmul(
                                    pq[:, s * 512:(s + 1) * 512],
                                    wqk[k][:, cc * 128:(cc + 1) * 128],
                                    xT[k][:, s * 512:(s + 1) * 512],
                                    start=(k == 0),
                                    stop=(k == 7),
                                )
                        yt = yqk_pool.tile(
                            [128, N], f32r, tag=tagn, name=f"{tagn}{p}"
                        )
                        nc.vector.tensor_scalar_add(yt[:], pq[:], bqk_t[:, cc:cc + 1])
                        ytiles[tagn] = yt
                    yq, yk = ytiles["yq"], ytiles["yk"]

                    # S^T + exp, heads 2p (base 0) and 2p+1 (base 64) row-packed
                    es = {}
                    for kc in range(8):
                        psj = [
                            ps.tile([128, N], f32, tag="s", name=f"ps{p}_{j}_{kc}")
                            for j in range(2)
                        ]
                        for qc in range(2):
                            for j, p0 in ((0, 0), (1, 64)):
                                nc.tensor.matmul(
                                    psj[j][:, qc * 512:(qc + 1) * 512],
                                    yk[p0:p0 + 64, kc * 128:(kc + 1) * 128],
                                    yq[p0:p0 + 64, qc * 512:(qc + 1) * 512],
                                    start=True,
                                    stop=True,
                                )
                        for j in range(2):
                            for qc in range(2):
                                e = es_pool.tile(
                                    [128, 512], f32r, tag="es",
                                    name=f"es{p}_{j}_{kc}_{qc}",
                                )
                                nc.scalar.activation(
                                    e[:],
                                    psj[j][:, qc * 512:(qc + 1) * 512],
                                    Act.Exp,
                                    scale=SCALE,
                                )
                                es[(j, kc, qc)] = e

                    # AV + normalize per head
                    for j, p0 in ((0, 0), (1, 64)):
                        h = 2 * p + j
                        for qc in range(2):
                            pav = psav.tile(
                                [65, 512], f32, tag="av", name=f"pav{h}_{qc}"
                            )
                            for kc in range(8):
                                nc.tensor.matmul(
                                    pav[:],
                                    vst[:, kc, h, :],
                                    es[(j, kc, qc)][:],
                                    start=(kc == 0),
                                    stop=(kc == 7),
                                )
                            rc = norm.tile([1, 512], f32, tag="rc", name=f"rc{h}{qc}")
                            nc.vector.reciprocal(rc[:], pav[64:65, :])
                            bc = norm.tile(
                                [64, 512], f32, tag="bc", name=f"bc{h}{qc}"
                            )
                            nc.gpsimd.partition_broadcast(bc[:], rc[0:1, :])
                            nc.vector.tensor_mul(
                                out=attr[p0:p0 + 64, p, qc * 512:(qc + 1) * 512],
                                in0=pav[0:64, :],
                                in1=bc[:],
                            )

            # ---- output projection ----
            with tc.tile_pool(name="proj", bufs=1) as proj:
                wp = [
                    proj.tile([128, 1024], f32r, tag=f"wp{c}", name=f"wp{c}")
                    for c in range(4)
                ]
                for c in range(4):
                    nc.gpsimd.dma_start(wp[c][:], wp_d[c * 128:(c + 1) * 128, :])
                with tc.tile_pool(name="yo_pool", bufs=3) as yo_pool:
                    for st in range(8):
                        po = ps.tile([128, N], f32, tag="s", name=f"po{st}")
                        for oc in range(2):
                            for c in range(4):
                                nc.tensor.matmul(
                                    po[:, oc * 512:(oc + 1) * 512],
                                    attr[:, c, st * 128:(st + 1) * 128],
                                    wp[c][:, oc * 512:(oc + 1) * 512],
                                    start=(c == 0),
                                    stop=(c == 3),
                                )
                        yo = yo_pool.tile([128, N], f32, tag="yo", name=f"yo{st}")
                        nc.vector.tensor_copy(yo[:], po[:])
                        nc.sync.dma_start(y_d[st * 128:(st + 1) * 128, :], yo[:])

    nc.compile()
    return nc


def _get_nc():
    if "nc" not in _NC_CACHE:
        _NC_CACHE["nc"] = _build_bass()
    return _NC_CACHE["nc"]


def _shard_inputs(x, w_qkv, b_qkv, w_proj):
    """Build per-core input maps. core = 2*b + hh."""
    ones64 = np.ones((128, 64), dtype=np.float32)

    in_maps = []
    for core in range(NCORES):
        b = core // 2
        hh = core % 2
        q_sl = slice(hh * 512, (hh + 1) * 512)
        k_sl = slice(1024 + hh * 512, 1024 + (hh + 1) * 512)
        v_sl = slice(2048 + hh * 512, 2048 + (hh + 1) * 512)

        xT = np.ascontiguousarray(x[b].T)
        wqk = np.ascontiguousarray(
            np.concatenate([w_qkv[:, q_sl], w_qkv[:, k_sl]], axis=1)
        )
        wv = np.ascontiguousarray(w_qkv[:, v_sl])
        wp = np.ascontiguousarray(w_proj[hh * 512:(hh + 1) * 512, :])
        bqk = np.ascontiguousarray(
            np.concatenate([b_qkv[q_sl], b_qkv[k_sl]]).reshape(8, 128).T
        )
        bv = np.ascontiguousarray(np.broadcast_to(b_qkv[v_sl], (128, 512)))
        in_maps.append(
            {
                "xT": xT,
                "wqk": wqk,
                "wv": wv,
                "wp": wp,
                "bqk": bqk,
                "bv": bv,
                "ones64": ones64,
            }
        )
    return in_maps


def kernel(x, w_qkv, b_qkv, w_proj, b_proj):
    from concourse.bass_utils import run_bass_kernel_spmd

    x = np.asarray(x, dtype=np.float32)
    w_qkv = np.asarray(w_qkv, dtype=np.float32)
    b_qkv = np.asarray(b_qkv, dtype=np.float32)
    w_proj = np.asarray(w_proj, dtype=np.float32)
    b_proj = np.asarray(b_proj, dtype=np.float32)

    nc = _get_nc()
    in_maps = _shard_inputs(x, w_qkv, b_qkv, w_proj)
    res = run_bass_kernel_spmd(nc, in_maps, core_ids=list(range(NCORES)))

    out = np.empty((B, N, C), dtype=np.float32)
    for b in range(B):
        out[b] = res.results[2 * b]["y"] + res.results[2 * b + 1]["y"]
    out += b_proj
    return out


# revision 13
# speedup vs baseline: 1.2390x; 1.2390x over previous
"""Multi-head attention block (B=4, N=1024, C=1024, H=16, d=64) on 8 TRN2 cores.

Sharding: core = 2*b + hh  (batch b in 0..3, head-half hh in 0..1 -> 8 heads/core).
Each core computes the qkv projection for its 8 heads, attention, and a partial
output projection (its 512 rows of w_proj). Host sums the two partials per
batch and adds b_proj.

Per-core pipeline (all matmul inputs float32r -> 1 PE cycle/row):
  - x^T fed from host, so Y_qk^T[cols,seq] = (W_qk chunks).T @ x^T chunks gives
    q^T,k^T directly; Y_v[seq,vcols] = (x^T chunks).T @ W_v gives V naturally.
    Zero on-chip transposes.
  - per head pair (heads 2p, 2p+1 at partition bases 0/64): S^T[keys,q] =
    k^T.T @ q^T with K=64; the two heads' matmuls target disjoint PE row
    groups and run concurrently. exp on ACT (scale=1/8 folded in), into f32r.
  - AV with V augmented by a ones column: one PSUM accumulation yields both
    att^T[64,q] and the softmax denominators (row 64). Normalize: DVE
    reciprocal (PSUM row), gpsimd partition_broadcast, DVE multiply -> att^T.
  - proj: out[seq,outfeat] = (att^T chunks).T @ w_proj chunks, DVE evict,
    DMA out. QKV production, attention, and eviction pipeline across engines;
    phases interleave per head pair.
"""

import numpy as np

B = 4
N = 1024
C = 1024
H = 16
D = 64
NCORES = 8
SCALE = D ** -0.5


_NC_CACHE = {}


def _build_bass():
    import concourse.mybir as mybir
    from concourse import bacc
    from concourse.tile import TileContext

    dt = mybir.dt
    f32 = dt.float32
    f32r = dt.float32r
    Act = mybir.ActivationFunctionType

    nc = bacc.Bacc(
        "TRN2",
        target_bir_lowering=False,
        debug=False,
        num_devices=NCORES,
        num_swdge_queues=4,
    )

    # ---- DRAM I/O (per-core shards; host prepares layouts) ----
    xT_d = nc.dram_tensor("xT", [C, N], f32, kind="ExternalInput").ap()
    wqk_d = nc.dram_tensor("wqk", [C, 1024], f32, kind="ExternalInput").ap()
    wv_d = nc.dram_tensor("wv", [C, 512], f32, kind="ExternalInput").ap()
    wp_d = nc.dram_tensor("wp", [512, C], f32, kind="ExternalInput").ap()
    bqk_d = nc.dram_tensor("bqk", [128, 8], f32, kind="ExternalInput").ap()
    bv_d = nc.dram_tensor("bv", [128, 512], f32, kind="ExternalInput").ap()
    ones_d = nc.dram_tensor("ones64", [128, 64], f32, kind="ExternalInput").ap()
    y_d = nc.dram_tensor("y", [N, C], f32, kind="ExternalOutput").ap()

    with TileContext(nc) as tc:
        with (
            tc.tile_pool(name="persist", bufs=1) as persist,
            tc.tile_pool(name="yqk_pool", bufs=3) as yqk_pool,
            tc.tile_pool(name="es_pool", bufs=20) as es_pool,
            tc.tile_pool(name="norm", bufs=3) as norm,
            tc.tile_pool(name="psum", bufs=3, space="PSUM") as ps,
            tc.tile_pool(name="psav", bufs=2, space="PSUM") as psav,
        ):
            # persistent SBUF tensors
            vst = persist.tile([128, 8, 8, 65], dt.bfloat16, tag="vst")  # [keys128, s, h, d+1]
            attr = persist.tile([128, 4, N], f32r, tag="attr")  # att^T normalized
            bqk_t = persist.tile([128, 8], f32, tag="bqk")
            bv_t = persist.tile([128, 512], f32, tag="bv")

            nc.sync.dma_start(bqk_t[:], bqk_d)
            nc.sync.dma_start(bv_t[:], bv_d)

            with tc.tile_pool(name="ph1", bufs=1) as ph1:
                xT = [
                    ph1.tile([128, N], f32r, tag=f"xT{k}", name=f"xT{k}")
                    for k in range(8)
                ]
                wqk = [
                    ph1.tile([128, 1024], f32r, tag=f"wqk{k}", name=f"wqk{k}")
                    for k in range(8)
                ]
                wv = [
                    ph1.tile([128, 512], f32r, tag=f"wv{k}", name=f"wv{k}")
                    for k in range(8)
                ]
                for k in range(8):
                    nc.gpsimd.dma_start(xT[k][:], xT_d[k * 128:(k + 1) * 128, :])
                    nc.gpsimd.dma_start(wv[k][:], wv_d[k * 128:(k + 1) * 128, :])
                    nc.gpsimd.dma_start(wqk[k][:], wqk_d[k * 128:(k + 1) * 128, :])
                # ones column of V-hat
                nc.gpsimd.dma_start(
                    vst[:, :, :, 64],
                    ones_d.rearrange("p (s h) -> p s h", s=8),
                )

                # ---- Y_v [seq, vcols] ----
                for s in range(8):
                    pv = ps.tile([128, 512], f32, tag="s", name=f"pv{s}")
                    for k in range(8):
                        nc.tensor.matmul(
                            pv[:],
                            xT[k][:, s * 128:(s + 1) * 128],
                            wv[k][:],
                            start=(k == 0),
                            stop=(k == 7),
                        )
                    nc.vector.tensor_add(
                        out=vst[:, s, :, 0:64],
                        in0=pv[:].rearrange("p (h d) -> p h d", h=8),
                        in1=bv_t[:].rearrange("p (h d) -> p h d", h=8),
                    )

                # ---- per head-pair software pipeline ----
                # stages per pair p: Yqk(p) -> S^T+exp(p) -> AV+norm(p).
                # Emit Yqk two pairs ahead and AV one pair behind so PE has
                # work (AV(p-1), Yqk(p+1)) while ACT runs exp(p).
                yqs, yks = {}, {}

                def emit_yqk(p):
                    for cc, tagn in ((p, "yq"), (4 + p, "yk")):
                        pq = ps.tile([128, N], f32, tag="s", name=f"pq{cc}")
                        for s in range(2):
                            for k in range(8):
                                nc.tensor.matmul(
                                    pq[:, s * 512:(s + 1) * 512],
                                    wqk[k][:, cc * 128:(cc + 1) * 128],
                                    xT[k][:, s * 512:(s + 1) * 512],
                                    start=(k == 0),
                                    stop=(k == 7),
                                )
                        yt = yqk_pool.tile(
                            [128, N], f32r, tag=tagn, name=f"{tagn}{p}"
                        )
                        nc.vector.tensor_scalar_add(yt[:], pq[:], bqk_t[:, cc:cc + 1])
                        (yqs if tagn == "yq" else yks)[p] = yt

                def emit_st_exp(p):
                    yq, yk = yqs[p], yks[p]
                    es = {}
                    for kc in range(8):
                        psj = [
                            ps.tile([128, N], f32, tag="s", name=f"ps{p}_{j}_{kc}")
                            for j in range(2)
                        ]
                        for qc in range(2):
                            for j, p0 in ((0, 0), (1, 64)):
                                nc.tensor.matmul(
                                    psj[j][:, qc * 512:(qc + 1) * 512],
                                    yk[p0:p0 + 64, kc * 128:(kc + 1) * 128],
                                    yq[p0:p0 + 64, qc * 512:(qc + 1) * 512],
                                    start=True,
                                    stop=True,
                                )
                        for j in range(2):
                            e = es_pool.tile(
                                [128, N], dt.bfloat16, tag="es",
                                name=f"es{p}_{j}_{kc}",
                            )
                            nc.scalar.activation(
                                e[:], psj[j][:], Act.Exp, scale=SCALE
                            )
                            es[(j, kc)] = e
                    return es

                def emit_av(p, es):
                    for j, p0 in ((0, 0), (1, 64)):
                        h = 2 * p + j
                        for qc in range(2):
                            pav = psav.tile(
                                [65, 512], f32, tag="av", name=f"pav{h}_{qc}"
                            )
                            for kc in range(8):
                                nc.tensor.matmul(
                                    pav[:],
                                    vst[:, kc, h, :],
                                    es[(j, kc)][:, qc * 512:(qc + 1) * 512],
                                    start=(kc == 0),
                                    stop=(kc == 7),
                                )
                            rc = norm.tile([1, 512], f32, tag="rc", name=f"rc{h}{qc}")
                            nc.vector.reciprocal(rc[:], pav[64:65, :])
                            bc = norm.tile(
                                [64, 512], f32, tag="bc", name=f"bc{h}{qc}"
                            )
                            nc.gpsimd.partition_broadcast(bc[:], rc[0:1, :])
                            nc.vector.tensor_mul(
                                out=attr[p0:p0 + 64, p, qc * 512:(qc + 1) * 512],
                                in0=pav[0:64, :],
                                in1=bc[:],
                            )

                emit_yqk(0)
                emit_yqk(1)
                prev = None
                for p in range(4):
                    es = emit_st_exp(p)
                    if prev is not None:
                        emit_av(*prev)
                    if p + 2 < 4:
                        emit_yqk(p + 2)
                    prev = (p, es)
                emit_av(*prev)

            # ---- output projection ----
            with tc.tile_pool(name="proj", bufs=1) as proj:
                wp = [
                    proj.tile([128, 1024], f32r, tag=f"wp{c}", name=f"wp{c}")
                    for c in range(4)
                ]
                for c in range(4):
                    nc.gpsimd.dma_start(wp[c][:], wp_d[c * 128:(c + 1) * 128, :])
                with tc.tile_pool(name="yo_pool", bufs=3) as yo_pool:
                    for st in range(8):
                        po = ps.tile([128, N], f32, tag="s", name=f"po{st}")
                        for oc in range(2):
                            for c in range(4):
                                nc.tensor.matmul(
                                    po[:, oc * 512:(oc + 1) * 512],
                                    attr[:, c, st * 128:(st + 1) * 128],
                                    wp[c][:, oc * 512:(oc + 1) * 512],
                                    start=(c == 0),
                                    stop=(c == 3),
                                )
                        yo = yo_pool.tile([128, N], f32, tag="yo", name=f"yo{st}")
                        nc.vector.tensor_copy(yo[:], po[:])
                        nc.sync.dma_start(y_d[st * 128:(st + 1) * 128, :], yo[:])

    nc.compile()
    return nc


def _get_nc():
    if "nc" not in _NC_CACHE:
        _NC_CACHE["nc"] = _build_bass()
    return _NC_CACHE["nc"]


def _shard_inputs(x, w_qkv, b_qkv, w_proj):
    """Build per-core input maps. core = 2*b + hh."""
    ones64 = np.ones((128, 64), dtype=np.float32)

    in_maps = []
    for core in range(NCORES):
        b = core // 2
        hh = core % 2
        q_sl = slice(hh * 512, (hh + 1) * 512)
        k_sl = slice(1024 + hh * 512, 1024 + (hh + 1) * 512)
        v_sl = slice(2048 + hh * 512, 2048 + (hh + 1) * 512)

        xT = np.ascontiguousarray(x[b].T)
        wqk = np.ascontiguousarray(
            np.concatenate([w_qkv[:, q_sl], w_qkv[:, k_sl]], axis=1)
        )
        wv = np.ascontiguousarray(w_qkv[:, v_sl])
        wp = np.ascontiguousarray(w_proj[hh * 512:(hh + 1) * 512, :])
        bqk = np.ascontiguousarray(
            np.concatenate([b_qkv[q_sl], b_qkv[k_sl]]).reshape(8, 128).T
        )
        bv = np.ascontiguousarray(np.broadcast_to(b_qkv[v_sl], (128, 512)))
        in_maps.append(
            {
                "xT": xT,
                "wqk": wqk,
                "wv": wv,
                "wp": wp,
                "bqk": bqk,
                "bv": bv,
                "ones64": ones64,
            }
        )
    return in_maps


def kernel(x, w_qkv, b_qkv, w_proj, b_proj):
    from concourse.bass_utils import run_bass_kernel_spmd

    x = np.asarray(x, dtype=np.float32)
    w_qkv = np.asarray(w_qkv, dtype=np.float32)
    b_qkv = np.asarray(b_qkv, dtype=np.float32)
    w_proj = np.asarray(w_proj, dtype=np.float32)
    b_proj = np.asarray(b_proj, dtype=np.float32)

    nc = _get_nc()
    in_maps = _shard_inputs(x, w_qkv, b_qkv, w_proj)
    res = run_bass_kernel_spmd(nc, in_maps, core_ids=list(range(NCORES)))

    out = np.empty((B, N, C), dtype=np.float32)
    for b in range(B):
        out[b] = res.results[2 * b]["y"] + res.results[2 * b + 1]["y"]
    out += b_proj
    return out


# revision 21
# speedup vs baseline: 1.3594x; 1.0971x over previous
"""Multi-head attention block (B=4, N=1024, C=1024, H=16, d=64) on 8 TRN2 cores.

Sharding: core = 2*b + hh  (batch b in 0..3, head-half hh in 0..1 -> 8 heads/core).
Each core computes the qkv projection for its 8 heads, attention, and a partial
output projection (its 512 rows of w_proj). Host sums the two partials per
batch and adds b_proj.

Per-core pipeline (all matmul inputs float32r -> 1 PE cycle/row):
  - x^T fed from host, so Y_qk^T[cols,seq] = (W_qk chunks).T @ x^T chunks gives
    q^T,k^T directly; Y_v[seq,vcols] = (x^T chunks).T @ W_v gives V naturally.
    Zero on-chip transposes.
  - per head pair (heads 2p, 2p+1 at partition bases 0/64): S^T[keys,q] =
    k^T.T @ q^T with K=64; the two heads' matmuls target disjoint PE row
    groups and run concurrently. exp on ACT (scale=1/8 folded in), into f32r.
  - AV with V augmented by a ones column: one PSUM accumulation yields both
    att^T[64,q] and the softmax denominators (row 64). Normalize: DVE
    reciprocal (PSUM row), gpsimd partition_broadcast, DVE multiply -> att^T.
  - proj: out[seq,outfeat] = (att^T chunks).T @ w_proj chunks, DVE evict,
    DMA out. QKV production, attention, and eviction pipeline across engines;
    phases interleave per head pair.
"""

import numpy as np

B = 4
N = 1024
C = 1024
H = 16
D = 64
NCORES = 8
SCALE = D ** -0.5


_NC_CACHE = {}


def _build_bass():
    import concourse.mybir as mybir
    from concourse import bacc
    from concourse.tile import TileContext

    dt = mybir.dt
    f32 = dt.float32
    f32r = dt.float32r
    Act = mybir.ActivationFunctionType

    nc = bacc.Bacc(
        "TRN2",
        target_bir_lowering=False,
        debug=False,
        num_devices=NCORES,
        num_swdge_queues=4,
    )

    # ---- DRAM I/O (per-core shards; host prepares layouts) ----
    xT_d = nc.dram_tensor("xT", [C, N], f32r, kind="ExternalInput").ap()
    wqk_d = nc.dram_tensor("wqk", [8, 128, 8, 128], f32r, kind="ExternalInput").ap()
    wv_d = nc.dram_tensor("wv", [C, 512], f32r, kind="ExternalInput").ap()
    wp_d = nc.dram_tensor("wp", [512, C], f32r, kind="ExternalInput").ap()
    bqk_d = nc.dram_tensor("bqk", [128, 8], f32, kind="ExternalInput").ap()
    bv_d = nc.dram_tensor("bv", [128, 512], f32, kind="ExternalInput").ap()
    ones_d = nc.dram_tensor("ones64", [128, 64], dt.bfloat16, kind="ExternalInput").ap()
    y_d = nc.dram_tensor("y", [N, C], f32, kind="ExternalOutput").ap()

    with TileContext(nc) as tc:
        with (
            tc.tile_pool(name="persist", bufs=1) as persist,
            tc.tile_pool(name="yqk_pool", bufs=3) as yqk_pool,
            tc.tile_pool(name="es_pool", bufs=20) as es_pool,
            tc.tile_pool(name="norm", bufs=3) as norm,
            tc.tile_pool(name="psum", bufs=3, space="PSUM") as ps,
            tc.tile_pool(name="psav", bufs=2, space="PSUM") as psav,
        ):
            # persistent SBUF tensors
            vst = persist.tile([128, 8, 8, 65], dt.bfloat16, tag="vst")  # [keys128, s, h, d+1]
            attr = persist.tile([128, 4, N], f32r, tag="attr")  # att^T normalized
            bqk_t = persist.tile([128, 8], f32, tag="bqk")
            bv_t = persist.tile([128, 512], f32, tag="bv")

            nc.sync.dma_start(bqk_t[:], bqk_d)
            nc.sync.dma_start(bv_t[:], bv_d)

            with tc.tile_pool(name="ph1", bufs=1) as ph1:
                xT = [
                    ph1.tile([128, N], f32r, tag=f"xT{k}", name=f"xT{k}")
                    for k in range(8)
                ]
                # wqk column strips [partition, ko, col]; host pre-tiled
                wqs = [
                    ph1.tile([128, 8, 128], f32r, tag=f"wq{cc}", name=f"wq{cc}")
                    for cc in range(8)
                ]
                wv = [
                    ph1.tile([128, 512], f32r, tag=f"wv{k}", name=f"wv{k}")
                    for k in range(8)
                ]
                # ones column of V-hat
                nc.sync.dma_start(
                    vst[:, :, :, 64],
                    ones_d.rearrange("p (s h) -> p s h", s=8),
                )
                # DMA priority: pair-0 attention inputs, then V, then the rest
                nc.sync.dma_start(wqs[0][:], wqk_d[0])
                nc.sync.dma_start(wqs[4][:], wqk_d[4])
                for k in range(8):
                    nc.sync.dma_start(xT[k][:], xT_d[k * 128:(k + 1) * 128, :])
                nc.sync.dma_start(wqs[1][:], wqk_d[1])
                nc.sync.dma_start(wqs[5][:], wqk_d[5])
                for k in range(8):
                    nc.sync.dma_start(wv[k][:], wv_d[k * 128:(k + 1) * 128, :])
                for p_ in range(2, 4):
                    nc.sync.dma_start(wqs[p_][:], wqk_d[p_])
                    nc.sync.dma_start(wqs[4 + p_][:], wqk_d[4 + p_])

                # ---- per head-pair software pipeline ----
                # Interleave: S^T+exp(p) runs on ACT while PE fills the gaps
                # with AV(p-1) groups (or Y_v for p=0) and the next pair's Yqk.
                yqs, yks = {}, {}

                def emit_yqk(p):
                    for cc, tagn in ((p, "yq"), (4 + p, "yk")):
                        pq = ps.tile([128, N], f32, tag="s", name=f"pq{cc}")
                        for s in range(2):
                            for k in range(8):
                                nc.tensor.matmul(
                                    pq[:, s * 512:(s + 1) * 512],
                                    wqs[cc][:, k, :],
                                    xT[k][:, s * 512:(s + 1) * 512],
                                    start=(k == 0),
                                    stop=(k == 7),
                                )
                        yt = yqk_pool.tile(
                            [128, N], f32r, tag=tagn, name=f"{tagn}{p}"
                        )
                        nc.vector.tensor_scalar_add(yt[:], pq[:], bqk_t[:, cc:cc + 1])
                        (yqs if tagn == "yq" else yks)[p] = yt

                def emit_yv_group(s):
                    pv = ps.tile([128, 512], f32, tag="s", name=f"pv{s}")
                    for k in range(8):
                        nc.tensor.matmul(
                            pv[:],
                            xT[k][:, s * 128:(s + 1) * 128],
                            wv[k][:],
                            start=(k == 0),
                            stop=(k == 7),
                        )
                    nc.vector.tensor_add(
                        out=vst[:, s, :, 0:64],
                        in0=pv[:].rearrange("p (h d) -> p h d", h=8),
                        in1=bv_t[:].rearrange("p (h d) -> p h d", h=8),
                    )

                def emit_av_group(p, es, j, qc):
                    p0 = j * 64
                    h = 2 * p + j
                    pav = psav.tile([65, 512], f32, tag="av", name=f"pav{h}_{qc}")
                    for kc in range(8):
                        nc.tensor.matmul(
                            pav[:],
                            vst[:, kc, h, :],
                            es[(j, kc)][:, qc * 512:(qc + 1) * 512],
                            start=(kc == 0),
                            stop=(kc == 7),
                        )
                    rc = norm.tile([1, 512], f32, tag="rc", name=f"rc{h}{qc}")
                    nc.vector.reciprocal(rc[:], pav[64:65, :])
                    bc = norm.tile([64, 512], f32, tag="bc", name=f"bc{h}{qc}")
                    nc.gpsimd.partition_broadcast(bc[:], rc[0:1, :])
                    nc.vector.tensor_mul(
                        out=attr[p0:p0 + 64, p, qc * 512:(qc + 1) * 512],
                        in0=pav[0:64, :],
                        in1=bc[:],
                    )

                def emit_st_exp(p, filler):
                    """S^T+exp for pair p; `filler(kc)` emits PE work to
                    overlap the ACT-paced exp stream."""
                    yq, yk = yqs[p], yks[p]
                    es = {}
                    for kc in range(8):
                        psj = [
                            ps.tile([128, N], f32, tag="s", name=f"ps{p}_{j}_{kc}")
                            for j in range(2)
                        ]
                        for qc in range(2):
                            for j, p0 in ((0, 0), (1, 64)):
                                nc.tensor.matmul(
                                    psj[j][:, qc * 512:(qc + 1) * 512],
                                    yk[p0:p0 + 64, kc * 128:(kc + 1) * 128],
                                    yq[p0:p0 + 64, qc * 512:(qc + 1) * 512],
                                    start=True,
                                    stop=True,
                                )
                        for j in range(2):
                            e = es_pool.tile(
                                [128, N], dt.bfloat16, tag="es",
                                name=f"es{p}_{j}_{kc}",
                            )
                            nc.scalar.activation(
                                e[:], psj[j][:], Act.Exp, scale=SCALE
                            )
                            es[(j, kc)] = e
                        filler(kc)
                    return es

                emit_yqk(0)
                emit_yqk(1)

                def filler0(kc):
                    if kc >= 2:
                        emit_yv_group(kc - 2)

                es_prev = emit_st_exp(0, filler0)
                emit_yv_group(6)
                emit_yv_group(7)
                prev_p = 0
                for p in range(1, 4):
                    avq = [(j, qc) for j in range(2) for qc in range(2)]

                    def filler(kc, _avq=avq, _pp=prev_p, _es=es_prev, _p=p):
                        if kc % 2 == 1 and _avq:
                            j, qc = _avq.pop(0)
                            emit_av_group(_pp, _es, j, qc)
                        if kc == 6 and _p + 1 < 4:
                            emit_yqk(_p + 1)

                    es_now = emit_st_exp(p, filler)
                    for j, qc in avq:
                        emit_av_group(prev_p, es_prev, j, qc)
                    es_prev, prev_p = es_now, p
                for j in range(2):
                    for qc in range(2):
                        emit_av_group(3, es_prev, j, qc)

            # ---- output projection ----
            with tc.tile_pool(name="proj", bufs=1) as proj:
                wp = [
                    proj.tile([128, 1024], f32r, tag=f"wp{c}", name=f"wp{c}")
                    for c in range(4)
                ]
                for c in range(4):
                    nc.sync.dma_start(wp[c][:], wp_d[c * 128:(c + 1) * 128, :])
                with tc.tile_pool(name="yo_pool", bufs=3) as yo_pool:
                    for st in range(8):
                        po = ps.tile([128, N], f32, tag="s", name=f"po{st}")
                        for oc in range(2):
                            for c in range(4):
                                nc.tensor.matmul(
                                    po[:, oc * 512:(oc + 1) * 512],
                                    attr[:, c, st * 128:(st + 1) * 128],
                                    wp[c][:, oc * 512:(oc + 1) * 512],
                                    start=(c == 0),
                                    stop=(c == 3),
                                )
                        yo = yo_pool.tile([128, N], f32, tag="yo", name=f"yo{st}")
                        nc.vector.tensor_copy(yo[:], po[:])
                        nc.sync.dma_start(y_d[st * 128:(st + 1) * 128, :], yo[:])

    nc.compile()
    return nc


def _get_nc():
    if "nc" not in _NC_CACHE:
        _NC_CACHE["nc"] = _build_bass()
    return _NC_CACHE["nc"]


def _shard_inputs(x, w_qkv, b_qkv, w_proj):
    """Build per-core input maps. core = 2*b + hh."""
    import ml_dtypes

    ones64 = np.ones((128, 64), dtype=ml_dtypes.bfloat16)

    in_maps = []
    for core in range(NCORES):
        b = core // 2
        hh = core % 2
        q_sl = slice(hh * 512, (hh + 1) * 512)
        k_sl = slice(1024 + hh * 512, 1024 + (hh + 1) * 512)
        v_sl = slice(2048 + hh * 512, 2048 + (hh + 1) * 512)

        xT = np.ascontiguousarray(x[b].T)
        wqk = np.concatenate([w_qkv[:, q_sl], w_qkv[:, k_sl]], axis=1)
        # [cc, p, ko, c]: tile (cc) is [partition, ko, col], contiguous per DMA
        wqk = np.ascontiguousarray(
            wqk.reshape(8, 128, 8, 128).transpose(2, 1, 0, 3)
        )
        wv = np.ascontiguousarray(w_qkv[:, v_sl])
        wp = np.ascontiguousarray(w_proj[hh * 512:(hh + 1) * 512, :])
        bqk = np.ascontiguousarray(
            np.concatenate([b_qkv[q_sl], b_qkv[k_sl]]).reshape(8, 128).T
        )
        bv = np.ascontiguousarray(np.broadcast_to(b_qkv[v_sl], (128, 512)))
        in_maps.append(
            {
                "xT": xT,
                "wqk": wqk,
                "wv": wv,
                "wp": wp,
                "bqk": bqk,
                "bv": bv,
                "ones64": ones64,
            }
        )
    return in_maps


def kernel(x, w_qkv, b_qkv, w_proj, b_proj):
    from concourse.bass_utils import run_bass_kernel_spmd

    x = np.asarray(x, dtype=np.float32)
    w_qkv = np.asarray(w_qkv, dtype=np.float32)
    b_qkv = np.asarray(b_qkv, dtype=np.float32)
    w_proj = np.asarray(w_proj, dtype=np.float32)
    b_proj = np.asarray(b_proj, dtype=np.float32)

    nc = _get_nc()
    in_maps = _shard_inputs(x, w_qkv, b_qkv, w_proj)
    res = run_bass_kernel_spmd(nc, in_maps, core_ids=list(range(NCORES)))

    out = np.empty((B, N, C), dtype=np.float32)
    for b in range(B):
        out[b] = res.results[2 * b]["y"] + res.results[2 * b + 1]["y"]
    out += b_proj
    return out


# revision 34
# speedup vs baseline: 1.4493x; 1.0662x over previous
"""Multi-head attention block (B=4, N=1024, C=1024, H=16, d=64) on 8 TRN2 cores.

Sharding: core = 2*b + hh  (batch b in 0..3, head-half hh in 0..1 -> 8 heads/core).
Each core computes the qkv projection for its 8 heads, attention, and a partial
output projection (its 512 rows of w_proj). Host sums the two partials per
batch and adds b_proj.

Per-core pipeline (all matmul inputs float32r -> 1 PE cycle/row):
  - x^T fed from host, so Y_qk^T[cols,seq] = (W_qk chunks).T @ x^T chunks gives
    q^T,k^T directly; Y_v[seq,vcols] = (x^T chunks).T @ W_v gives V naturally.
    Zero on-chip transposes.
  - per head pair (heads 2p, 2p+1 at partition bases 0/64): S^T[keys,q] =
    k^T.T @ q^T with K=64; the two heads' matmuls target disjoint PE row
    groups and run concurrently. exp on ACT (scale=1/8 folded in), into f32r.
  - AV with V augmented by a ones column: one PSUM accumulation yields both
    att^T[64,q] and the softmax denominators (row 64). Normalize: DVE
    reciprocal (PSUM row), gpsimd partition_broadcast, DVE multiply -> att^T.
  - proj: out[seq,outfeat] = (att^T chunks).T @ w_proj chunks, DVE evict,
    DMA out. QKV production, attention, and eviction pipeline across engines;
    phases interleave per head pair.
"""

import numpy as np

B = 4
N = 1024
C = 1024
H = 16
D = 64
NCORES = 8
SCALE = D ** -0.5


_NC_CACHE = {}


def _build_bass():
    import concourse.mybir as mybir
    from concourse import bacc
    from concourse.tile import TileContext

    dt = mybir.dt
    f32 = dt.float32
    f32r = dt.float32r
    Act = mybir.ActivationFunctionType

    nc = bacc.Bacc(
        "TRN2",
        target_bir_lowering=False,
        debug=False,
        num_devices=NCORES,
        num_swdge_queues=4,
    )

    # ---- DRAM I/O (per-core shards; host prepares layouts) ----
    xT_d = nc.dram_tensor("xT", [C, N], f32r, kind="ExternalInput").ap()
    wqk_d = nc.dram_tensor("wqk", [8, 128, 8, 128], f32r, kind="ExternalInput").ap()
    wv_d = nc.dram_tensor("wv", [C, 512], f32r, kind="ExternalInput").ap()
    wp_d = nc.dram_tensor("wp", [512, C], f32r, kind="ExternalInput").ap()
    bqk_d = nc.dram_tensor("bqk", [128, 8], f32, kind="ExternalInput").ap()
    bv_d = nc.dram_tensor("bv", [128, 512], f32, kind="ExternalInput").ap()
    ones_d = nc.dram_tensor("ones64", [128, 64], dt.bfloat16, kind="ExternalInput").ap()
    y_d = nc.dram_tensor("y", [N, C], f32, kind="ExternalOutput").ap()

    with TileContext(nc) as tc:
        with (
            tc.tile_pool(name="persist", bufs=1) as persist,
            tc.tile_pool(name="yqk_pool", bufs=3) as yqk_pool,
            tc.tile_pool(name="es_pool", bufs=20) as es_pool,
            tc.tile_pool(name="norm", bufs=3) as norm,
            tc.tile_pool(name="psum", bufs=2, space="PSUM") as ps,
            tc.tile_pool(name="psum_sm", bufs=2, space="PSUM") as ps_sm,
            tc.tile_pool(name="psav", bufs=2, space="PSUM") as psav,
        ):
            # persistent SBUF tensors
            vst = persist.tile([128, 8, 8, 65], dt.bfloat16, tag="vst")  # [keys128, s, h, d+1]
            attr = [
                persist.tile([128, N], f32r, tag=f"attr{c}", name=f"attr{c}")
                for c in range(4)
            ]  # att^T normalized, per 2-head chunk
            bqk_t = persist.tile([128, 8], f32, tag="bqk")
            bv_t = persist.tile([128, 512], f32, tag="bv")

            nc.gpsimd.dma_start(bqk_t[:], bqk_d)
            nc.gpsimd.dma_start(bv_t[:], bv_d)

            with tc.tile_pool(name="ph1", bufs=1) as ph1:
                xT = [
                    ph1.tile([128, N], f32r, tag=f"xT{k}", name=f"xT{k}")
                    for k in range(8)
                ]
                # wqk column strips [partition, ko, col]; host pre-tiled
                wqs = [
                    ph1.tile([128, 8, 128], f32r, tag=f"wq{cc}", name=f"wq{cc}")
                    for cc in range(8)
                ]
                wv = [
                    ph1.tile([128, 512], f32r, tag=f"wv{k}", name=f"wv{k}")
                    for k in range(8)
                ]
                # ones column of V-hat
                nc.vector.memset(vst[:, :, :, 64], 1.0)
                # DMA priority: pair-0 attention inputs, then V, then the rest
                nc.sync.dma_start(wqs[0][:], wqk_d[0])
                for k in range(4):
                    nc.sync.dma_start(xT[k][:], xT_d[k * 128:(k + 1) * 128, :])
                nc.sync.dma_start(wqs[4][:], wqk_d[4])
                for k in range(4, 8):
                    nc.sync.dma_start(xT[k][:], xT_d[k * 128:(k + 1) * 128, :])
                nc.sync.dma_start(wqs[1][:], wqk_d[1])
                nc.sync.dma_start(wqs[5][:], wqk_d[5])
                for k in range(8):
                    nc.sync.dma_start(wv[k][:], wv_d[k * 128:(k + 1) * 128, :])
                for p_ in range(2, 4):
                    nc.sync.dma_start(wqs[p_][:], wqk_d[p_])
                    nc.sync.dma_start(wqs[4 + p_][:], wqk_d[4 + p_])

                # ---- per head-pair software pipeline ----
                # Interleave: S^T+exp(p) runs on ACT while PE fills the gaps
                # with AV(p-1) groups (or Y_v for p=0) and the next pair's Yqk.
                yqs, yks = {}, {}

                def emit_yqk(p):
                    for cc, tagn in ((p, "yq"), (4 + p, "yk")):
                        yt = yqk_pool.tile(
                            [128, N], f32r, tag=tagn, name=f"{tagn}{p}"
                        )
                        for s in range(2):
                            pq = ps_sm.tile(
                                [128, 512], f32, tag="sm", name=f"pq{cc}_{s}"
                            )
                            for k in range(8):
                                nc.tensor.matmul(
                                    pq[:],
                                    wqs[cc][:, k, :],
                                    xT[k][:, s * 512:(s + 1) * 512],
                                    start=(k == 0),
                                    stop=(k == 7),
                                )
                            nc.vector.tensor_scalar_add(
                                yt[:, s * 512:(s + 1) * 512],
                                pq[:],
                                bqk_t[:, cc:cc + 1],
                            )
                        (yqs if tagn == "yq" else yks)[p] = yt

                def emit_yv_group(s):
                    pv = ps_sm.tile([128, 512], f32, tag="sm", name=f"pv{s}")
                    for k in range(8):
                        nc.tensor.matmul(
                            pv[:],
                            xT[k][:, s * 128:(s + 1) * 128],
                            wv[k][:],
                            start=(k == 0),
                            stop=(k == 7),
                        )
                    nc.vector.tensor_add(
                        out=vst[:, s, :, 0:64],
                        in0=pv[:].rearrange("p (h d) -> p h d", h=8),
                        in1=bv_t[:].rearrange("p (h d) -> p h d", h=8),
                    )

                def emit_av_group(p, es, j, qc):
                    p0 = j * 64
                    h = 2 * p + j
                    pav = psav.tile([65, 512], f32, tag="av", name=f"pav{h}_{qc}")
                    for kc in range(8):
                        nc.tensor.matmul(
                            pav[:],
                            vst[:, kc, h, :],
                            es[(j, kc)][:, qc * 512:(qc + 1) * 512],
                            start=(kc == 0),
                            stop=(kc == 7),
                        )
                    rc = norm.tile([1, 512], f32, tag="rc", name=f"rc{h}{qc}")
                    nc.vector.reciprocal(rc[:], pav[64:65, :])
                    bc = norm.tile([64, 512], f32, tag="bc", name=f"bc{h}{qc}")
                    nc.gpsimd.partition_broadcast(bc[:], rc[0:1, :])
                    nc.vector.tensor_mul(
                        out=attr[p][p0:p0 + 64, qc * 512:(qc + 1) * 512],
                        in0=pav[0:64, :],
                        in1=bc[:],
                    )

                def emit_st_exp(p, filler):
                    """S^T+exp for pair p; `filler(kc)` emits PE work to
                    overlap the ACT-paced exp stream."""
                    yq, yk = yqs[p], yks[p]
                    es = {}
                    for kc in range(8):
                        psj = [
                            ps.tile([128, N], f32, tag="s", name=f"ps{p}_{j}_{kc}")
                            for j in range(2)
                        ]
                        for qc in range(2):
                            for j, p0 in ((0, 0), (1, 64)):
                                nc.tensor.matmul(
                                    psj[j][:, qc * 512:(qc + 1) * 512],
                                    yk[p0:p0 + 64, kc * 128:(kc + 1) * 128],
                                    yq[p0:p0 + 64, qc * 512:(qc + 1) * 512],
                                    start=True,
                                    stop=True,
                                )
                        for j in range(2):
                            e = es_pool.tile(
                                [128, N], dt.bfloat16, tag="es",
                                name=f"es{p}_{j}_{kc}",
                            )
                            nc.scalar.activation(
                                e[:], psj[j][:], Act.Exp, scale=SCALE
                            )
                            es[(j, kc)] = e
                        filler(kc)
                    return es

                emit_yqk(0)

                def filler0(kc):
                    if kc == 0:
                        emit_yqk(1)
                    if kc >= 2:
                        emit_yv_group(kc - 2)

                es_prev = emit_st_exp(0, filler0)
                emit_yv_group(6)
                emit_yv_group(7)
                prev_p = 0
                for p in range(1, 4):
                    avq = [(j, qc) for j in range(2) for qc in range(2)]

                    def filler(kc, _avq=avq, _pp=prev_p, _es=es_prev, _p=p):
                        if kc % 2 == 1 and _avq:
                            j, qc = _avq.pop(0)
                            emit_av_group(_pp, _es, j, qc)
                        if kc == 4 and _p + 1 < 4:
                            emit_yqk(_p + 1)

                    es_now = emit_st_exp(p, filler)
                    for j, qc in avq:
                        emit_av_group(prev_p, es_prev, j, qc)
                    es_prev, prev_p = es_now, p
                for j in range(2):
                    for qc in range(2):
                        emit_av_group(3, es_prev, j, qc)

            # ---- output projection ----
            with tc.tile_pool(name="proj", bufs=1) as proj:
                wp = [
                    proj.tile([128, 1024], f32r, tag=f"wp{c}", name=f"wp{c}")
                    for c in range(4)
                ]
                for c in range(4):
                    nc.sync.dma_start(wp[c][:], wp_d[c * 128:(c + 1) * 128, :])
                with tc.tile_pool(name="yo_pool", bufs=3) as yo_pool:
                    for st in range(8):
                        yo = yo_pool.tile([128, N], f32, tag="yo", name=f"yo{st}")
                        for oc in range(2):
                            pool_sel = ps_sm if (st + oc) % 2 == 0 else ps
                            po = pool_sel.tile(
                                [128, 512], f32,
                                tag="sm" if (st + oc) % 2 == 0 else "s",
                                name=f"po{st}_{oc}",
                            )
                            for c in range(4):
                                nc.tensor.matmul(
                                    po[:],
                                    attr[c][:, st * 128:(st + 1) * 128],
                                    wp[c][:, oc * 512:(oc + 1) * 512],
                                    start=(c == 0),
                                    stop=(c == 3),
                                )
                            nc.vector.tensor_copy(
                                yo[:, oc * 512:(oc + 1) * 512], po[:]
                            )
                        nc.sync.dma_start(y_d[st * 128:(st + 1) * 128, :], yo[:])

    nc.compile()
    return nc


def _get_nc():
    if "nc" not in _NC_CACHE:
        _NC_CACHE["nc"] = _build_bass()
    return _NC_CACHE["nc"]


def _shard_inputs(x, w_qkv, b_qkv, w_proj):
    """Build per-core input maps. core = 2*b + hh."""
    import ml_dtypes

    ones64 = np.ones((128, 64), dtype=ml_dtypes.bfloat16)

    in_maps = []
    for core in range(NCORES):
        b = core // 2
        hh = core % 2
        q_sl = slice(hh * 512, (hh + 1) * 512)
        k_sl = slice(1024 + hh * 512, 1024 + (hh + 1) * 512)
        v_sl = slice(2048 + hh * 512, 2048 + (hh + 1) * 512)

        xT = np.ascontiguousarray(x[b].T)
        wqk = np.concatenate([w_qkv[:, q_sl], w_qkv[:, k_sl]], axis=1)
        # [cc, p, ko, c]: tile (cc) is [partition, ko, col], contiguous per DMA
        wqk = np.ascontiguousarray(
            wqk.reshape(8, 128, 8, 128).transpose(2, 1, 0, 3)
        )
        wv = np.ascontiguousarray(w_qkv[:, v_sl])
        wp = np.ascontiguousarray(w_proj[hh * 512:(hh + 1) * 512, :])
        bqk = np.ascontiguousarray(
            np.concatenate([b_qkv[q_sl], b_qkv[k_sl]]).reshape(8, 128).T
        )
        bv = np.ascontiguousarray(np.broadcast_to(b_qkv[v_sl], (128, 512)))
        in_maps.append(
            {
                "xT": xT,
                "wqk": wqk,
                "wv": wv,
                "wp": wp,
                "bqk": bqk,
                "bv": bv,
                "ones64": ones64,
            }
        )
    return in_maps


def kernel(x, w_qkv, b_qkv, w_proj, b_proj):
    from concourse.bass_utils import run_bass_kernel_spmd

    x = np.asarray(x, dtype=np.float32)
    w_qkv = np.asarray(w_qkv, dtype=np.float32)
    b_qkv = np.asarray(b_qkv, dtype=np.float32)
    w_proj = np.asarray(w_proj, dtype=np.float32)
    b_proj = np.asarray(b_proj, dtype=np.float32)

    nc = _get_nc()
    in_maps = _shard_inputs(x, w_qkv, b_qkv, w_proj)
    res = run_bass_kernel_spmd(nc, in_maps, core_ids=list(range(NCORES)))

    out = np.empty((B, N, C), dtype=np.float32)
    for b in range(B):
        out[b] = res.results[2 * b]["y"] + res.results[2 * b + 1]["y"]
    out += b_proj
    return out


# revision 41
# speedup vs baseline: 1.4600x; 1.0074x over previous
"""Multi-head attention block (B=4, N=1024, C=1024, H=16, d=64) on 8 TRN2 cores.

Sharding: core = 2*b + hh  (batch b in 0..3, head-half hh in 0..1 -> 8 heads/core).
Each core computes the qkv projection for its 8 heads, attention, and a partial
output projection (its 512 rows of w_proj). Host sums the two partials per
batch and adds b_proj.

Per-core pipeline (all matmul inputs float32r -> 1 PE cycle/row):
  - x^T fed from host, so Y_qk^T[cols,seq] = (W_qk chunks).T @ x^T chunks gives
    q^T,k^T directly; Y_v[seq,vcols] = (x^T chunks).T @ W_v gives V naturally.
    Zero on-chip transposes.
  - per head pair (heads 2p, 2p+1 at partition bases 0/64): S^T[keys,q] =
    k^T.T @ q^T with K=64; the two heads' matmuls target disjoint PE row
    groups and run concurrently. exp on ACT (scale=1/8 folded in), into f32r.
  - AV with V augmented by a ones column: one PSUM accumulation yields both
    att^T[64,q] and the softmax denominators (row 64). Normalize: DVE
    reciprocal (PSUM row), gpsimd partition_broadcast, DVE multiply -> att^T.
  - proj: out[seq,outfeat] = (att^T chunks).T @ w_proj chunks, DVE evict,
    DMA out. QKV production, attention, and eviction pipeline across engines;
    phases interleave per head pair.
"""

import numpy as np

B = 4
N = 1024
C = 1024
H = 16
D = 64
NCORES = 8
SCALE = D ** -0.5


_NC_CACHE = {}


def _build_bass():
    import concourse.mybir as mybir
    from concourse import bacc
    from concourse.tile import TileContext

    dt = mybir.dt
    f32 = dt.float32
    f32r = dt.float32r
    Act = mybir.ActivationFunctionType

    nc = bacc.Bacc(
        "TRN2",
        target_bir_lowering=False,
        debug=False,
        num_devices=NCORES,
        num_swdge_queues=4,
    )

    # ---- DRAM I/O (per-core shards; host prepares layouts) ----
    xT_d = nc.dram_tensor("xT", [C, N], f32r, kind="ExternalInput").ap()
    wqk_d = nc.dram_tensor("wqk", [8, 128, 8, 128], f32r, kind="ExternalInput").ap()
    wv_d = nc.dram_tensor("wv", [C, 512], f32r, kind="ExternalInput").ap()
    wp_d = nc.dram_tensor("wp", [512, C], f32r, kind="ExternalInput").ap()
    bqk_d = nc.dram_tensor("bqk", [128, 8], f32, kind="ExternalInput").ap()
    bv_d = nc.dram_tensor("bv", [128, 512], f32, kind="ExternalInput").ap()
    ones_d = nc.dram_tensor("ones64", [128, 64], dt.bfloat16, kind="ExternalInput").ap()
    y_d = nc.dram_tensor("y", [N, C], f32, kind="ExternalOutput").ap()

    with TileContext(nc) as tc:
        with (
            tc.tile_pool(name="persist", bufs=1) as persist,
            tc.tile_pool(name="yqk_pool", bufs=3) as yqk_pool,
            tc.tile_pool(name="es_pool", bufs=22) as es_pool,
            tc.tile_pool(name="norm", bufs=3) as norm,
            tc.tile_pool(name="psum", bufs=2, space="PSUM") as ps,
            tc.tile_pool(name="psum_sm", bufs=2, space="PSUM") as ps_sm,
            tc.tile_pool(name="psav", bufs=2, space="PSUM") as psav,
        ):
            # persistent SBUF tensors
            vst = persist.tile([128, 8, 8, 65], dt.bfloat16, tag="vst")  # [keys128, s, h, d+1]
            attr = [
                persist.tile([128, N], f32r, tag=f"attr{c}", name=f"attr{c}")
                for c in range(4)
            ]  # att^T normalized, per 2-head chunk
            bqk_t = persist.tile([128, 8], f32, tag="bqk")
            bv_t = persist.tile([128, 512], f32, tag="bv")

            nc.gpsimd.dma_start(bqk_t[:], bqk_d)
            nc.gpsimd.dma_start(bv_t[:], bv_d)

            with tc.tile_pool(name="ph1", bufs=1) as ph1:
                xT = [
                    ph1.tile([128, N], f32r, tag=f"xT{k}", name=f"xT{k}")
                    for k in range(8)
                ]
                # wqk column strips [partition, ko, col]; host pre-tiled
                wqs = [
                    ph1.tile([128, 8, 128], f32r, tag=f"wq{cc}", name=f"wq{cc}")
                    for cc in range(8)
                ]
                wv = [
                    ph1.tile([128, 512], f32r, tag=f"wv{k}", name=f"wv{k}")
                    for k in range(8)
                ]
                # ones column of V-hat
                nc.vector.memset(vst[:, :, :, 64], 1.0)
                # DMA priority: pair-0 attention inputs, then V, then the rest
                nc.sync.dma_start(wqs[0][:, 0:4, :], wqk_d[0, :, 0:4, :])
                nc.sync.dma_start(wqs[0][:, 4:8, :], wqk_d[0, :, 4:8, :])
                for k in range(4):
                    nc.sync.dma_start(xT[k][:], xT_d[k * 128:(k + 1) * 128, :])
                nc.sync.dma_start(wqs[4][:], wqk_d[4])
                for k in range(4, 8):
                    nc.sync.dma_start(xT[k][:], xT_d[k * 128:(k + 1) * 128, :])
                nc.sync.dma_start(wqs[1][:], wqk_d[1])
                nc.sync.dma_start(wqs[5][:], wqk_d[5])
                for k in range(8):
                    nc.sync.dma_start(wv[k][:], wv_d[k * 128:(k + 1) * 128, :])
                for p_ in range(2, 4):
                    nc.sync.dma_start(wqs[p_][:], wqk_d[p_])
                    nc.sync.dma_start(wqs[4 + p_][:], wqk_d[4 + p_])

                # ---- per head-pair software pipeline ----
                # Interleave: S^T+exp(p) runs on ACT while PE fills the gaps
                # with AV(p-1) groups (or Y_v for p=0) and the next pair's Yqk.
                yqs, yks = {}, {}

                def emit_yqk(p):
                    for cc, tagn in ((p, "yq"), (4 + p, "yk")):
                        yt = yqk_pool.tile(
                            [128, N], f32r, tag=tagn, name=f"{tagn}{p}"
                        )
                        for s in range(2):
                            pq = ps_sm.tile(
                                [128, 512], f32, tag="sm", name=f"pq{cc}_{s}"
                            )
                            for k in range(8):
                                nc.tensor.matmul(
                                    pq[:],
                                    wqs[cc][:, k, :],
                                    xT[k][:, s * 512:(s + 1) * 512],
                                    start=(k == 0),
                                    stop=(k == 7),
                                )
                            nc.vector.tensor_scalar_add(
                                yt[:, s * 512:(s + 1) * 512],
                                pq[:],
                                bqk_t[:, cc:cc + 1],
                            )
                        (yqs if tagn == "yq" else yks)[p] = yt

                def emit_yv_group(s):
                    pv = ps_sm.tile([128, 512], f32, tag="sm", name=f"pv{s}")
                    for k in range(8):
                        nc.tensor.matmul(
                            pv[:],
                            xT[k][:, s * 128:(s + 1) * 128],
                            wv[k][:],
                            start=(k == 0),
                            stop=(k == 7),
                        )
                    nc.vector.tensor_add(
                        out=vst[:, s, :, 0:64],
                        in0=pv[:].rearrange("p (h d) -> p h d", h=8),
                        in1=bv_t[:].rearrange("p (h d) -> p h d", h=8),
                    )

                def emit_av_group(p, es, j, qc):
                    p0 = j * 64
                    h = 2 * p + j
                    pav = psav.tile([65, 512], f32, tag="av", name=f"pav{h}_{qc}")
                    for kc in range(8):
                        nc.tensor.matmul(
                            pav[:],
                            vst[:, kc, h, :],
                            es[(j, kc)][:, qc * 512:(qc + 1) * 512],
                            start=(kc == 0),
                            stop=(kc == 7),
                        )
                    rc = norm.tile([1, 512], f32, tag="rc", name=f"rc{h}{qc}")
                    nc.vector.reciprocal(rc[:], pav[64:65, :])
                    bc = norm.tile([64, 512], f32, tag="bc", name=f"bc{h}{qc}")
                    nc.gpsimd.partition_broadcast(bc[:], rc[0:1, :])
                    nc.vector.tensor_mul(
                        out=attr[p][p0:p0 + 64, qc * 512:(qc + 1) * 512],
                        in0=pav[0:64, :],
                        in1=bc[:],
                    )

                def emit_st_exp(p, filler):
                    """S^T+exp for pair p; `filler(kc)` emits PE work to
                    overlap the ACT-paced exp stream."""
                    yq, yk = yqs[p], yks[p]
                    es = {}
                    for kc in range(8):
                        psj = [
                            ps.tile([128, N], f32, tag="s", name=f"ps{p}_{j}_{kc}")
                            for j in range(2)
                        ]
                        for qc in range(2):
                            for j, p0 in ((0, 0), (1, 64)):
                                nc.tensor.matmul(
                                    psj[j][:, qc * 512:(qc + 1) * 512],
                                    yk[p0:p0 + 64, kc * 128:(kc + 1) * 128],
                                    yq[p0:p0 + 64, qc * 512:(qc + 1) * 512],
                                    start=True,
                                    stop=True,
                                )
                        for j in range(2):
                            e = es_pool.tile(
                                [128, N], dt.bfloat16, tag="es",
                                name=f"es{p}_{j}_{kc}",
                            )
                            nc.scalar.activation(
                                e[:], psj[j][:], Act.Exp, scale=SCALE
                            )
                            es[(j, kc)] = e
                        filler(kc)
                    return es

                emit_yqk(0)

                def filler0(kc):
                    if kc == 0:
                        emit_yqk(1)
                    if kc >= 2:
                        emit_yv_group(kc - 2)

                es_prev = emit_st_exp(0, filler0)
                emit_yv_group(6)
                emit_yv_group(7)
                prev_p = 0
                for p in range(1, 4):
                    avq = [(j, qc) for j in range(2) for qc in range(2)]

                    def filler(kc, _avq=avq, _pp=prev_p, _es=es_prev, _p=p):
                        if kc % 2 == 1 and _avq:
                            j, qc = _avq.pop(0)
                            emit_av_group(_pp, _es, j, qc)
                        if kc == 4 and _p + 1 < 4:
                            emit_yqk(_p + 1)

                    es_now = emit_st_exp(p, filler)
                    for j, qc in avq:
                        emit_av_group(prev_p, es_prev, j, qc)
                    es_prev, prev_p = es_now, p
                for j in range(2):
                    for qc in range(2):
                        emit_av_group(3, es_prev, j, qc)

            # ---- output projection ----
            with tc.tile_pool(name="proj", bufs=1) as proj:
                wp = [
                    proj.tile([128, 1024], f32r, tag=f"wp{c}", name=f"wp{c}")
                    for c in range(4)
                ]
                for c in range(4):
                    nc.sync.dma_start(wp[c][:], wp_d[c * 128:(c + 1) * 128, :])
                with tc.tile_pool(name="yo_pool", bufs=3) as yo_pool:
                    for st in range(8):
                        yo = yo_pool.tile([128, N], f32, tag="yo", name=f"yo{st}")
                        for oc in range(2):
                            pool_sel = ps_sm if (st + oc) % 2 == 0 else ps
                            po = pool_sel.tile(
                                [128, 512], f32,
                                tag="sm" if (st + oc) % 2 == 0 else "s",
                                name=f"po{st}_{oc}",
                            )
                            for c in range(4):
                                nc.tensor.matmul(
                                    po[:],
                                    attr[c][:, st * 128:(st + 1) * 128],
                                    wp[c][:, oc * 512:(oc + 1) * 512],
                                    start=(c == 0),
                                    stop=(c == 3),
                                )
                            nc.vector.tensor_copy(
                                yo[:, oc * 512:(oc + 1) * 512], po[:]
                            )
                        nc.sync.dma_start(y_d[st * 128:(st + 1) * 128, :], yo[:])

    nc.compile()
    return nc


def _get_nc():
    if "nc" not in _NC_CACHE:
        _NC_CACHE["nc"] = _build_bass()
    return _NC_CACHE["nc"]


def _shard_inputs(x, w_qkv, b_qkv, w_proj):
    """Build per-core input maps. core = 2*b + hh."""
    import ml_dtypes

    ones64 = np.ones((128, 64), dtype=ml_dtypes.bfloat16)

    in_maps = []
    for core in range(NCORES):
        b = core // 2
        hh = core % 2
        q_sl = slice(hh * 512, (hh + 1) * 512)
        k_sl = slice(1024 + hh * 512, 1024 + (hh + 1) * 512)
        v_sl = slice(2048 + hh * 512, 2048 + (hh + 1) * 512)

        xT = np.ascontiguousarray(x[b].T)
        wqk = np.concatenate([w_qkv[:, q_sl], w_qkv[:, k_sl]], axis=1)
        # [cc, p, ko, c]: tile (cc) is [partition, ko, col], contiguous per DMA
        wqk = np.ascontiguousarray(
            wqk.reshape(8, 128, 8, 128).transpose(2, 1, 0, 3)
        )
        wv = np.ascontiguousarray(w_qkv[:, v_sl])
        wp = np.ascontiguousarray(w_proj[hh * 512:(hh + 1) * 512, :])
        bqk = np.ascontiguousarray(
            np.concatenate([b_qkv[q_sl], b_qkv[k_sl]]).reshape(8, 128).T
        )
        bv = np.ascontiguousarray(np.broadcast_to(b_qkv[v_sl], (128, 512)))
        in_maps.append(
            {
                "xT": xT,
                "wqk": wqk,
                "wv": wv,
                "wp": wp,
                "bqk": bqk,
                "bv": bv,
                "ones64": ones64,
            }
        )
    return in_maps


def kernel(x, w_qkv, b_qkv, w_proj, b_proj):
    from concourse.bass_utils import run_bass_kernel_spmd

    x = np.asarray(x, dtype=np.float32)
    w_qkv = np.asarray(w_qkv, dtype=np.float32)
    b_qkv = np.asarray(b_qkv, dtype=np.float32)
    w_proj = np.asarray(w_proj, dtype=np.float32)
    b_proj = np.asarray(b_proj, dtype=np.float32)

    nc = _get_nc()
    in_maps = _shard_inputs(x, w_qkv, b_qkv, w_proj)
    res = run_bass_kernel_spmd(nc, in_maps, core_ids=list(range(NCORES)))

    out = np.empty((B, N, C), dtype=np.float32)
    for b in range(B):
        out[b] = res.results[2 * b]["y"] + res.results[2 * b + 1]["y"]
    out += b_proj
    return out


# revision 44
# speedup vs baseline: 1.4717x; 1.0080x over previous
"""Multi-head attention block (B=4, N=1024, C=1024, H=16, d=64) on 8 TRN2 cores.

Sharding: core = 2*b + hh  (batch b in 0..3, head-half hh in 0..1 -> 8 heads/core).
Each core computes the qkv projection for its 8 heads, attention, and a partial
output projection (its 512 rows of w_proj). Host sums the two partials per
batch and adds b_proj.

Per-core pipeline (all matmul inputs float32r -> 1 PE cycle/row):
  - x^T fed from host, so Y_qk^T[cols,seq] = (W_qk chunks).T @ x^T chunks gives
    q^T,k^T directly; Y_v[seq,vcols] = (x^T chunks).T @ W_v gives V naturally.
    Zero on-chip transposes.
  - per head pair (heads 2p, 2p+1 at partition bases 0/64): S^T[keys,q] =
    k^T.T @ q^T with K=64; the two heads' matmuls target disjoint PE row
    groups and run concurrently. exp on ACT (scale=1/8 folded in), into f32r.
  - AV with V augmented by a ones column: one PSUM accumulation yields both
    att^T[64,q] and the softmax denominators (row 64). Normalize: DVE
    reciprocal (PSUM row), gpsimd partition_broadcast, DVE multiply -> att^T.
  - proj: out[seq,outfeat] = (att^T chunks).T @ w_proj chunks, DVE evict,
    DMA out. QKV production, attention, and eviction pipeline across engines;
    phases interleave per head pair.
"""

import numpy as np

B = 4
N = 1024
C = 1024
H = 16
D = 64
NCORES = 8
SCALE = D ** -0.5


_NC_CACHE = {}


def _build_bass():
    import concourse.mybir as mybir
    from concourse import bacc
    from concourse.tile import TileContext

    dt = mybir.dt
    f32 = dt.float32
    f32r = dt.float32r
    Act = mybir.ActivationFunctionType

    nc = bacc.Bacc(
        "TRN2",
        target_bir_lowering=False,
        debug=False,
        num_devices=NCORES,
        num_swdge_queues=4,
    )

    # ---- DRAM I/O (per-core shards; host prepares layouts) ----
    xT_d = nc.dram_tensor("xT", [C, N], f32r, kind="ExternalInput").ap()
    wqk_d = nc.dram_tensor("wqk", [8, 128, 8, 128], f32r, kind="ExternalInput").ap()
    wv_d = nc.dram_tensor("wv", [C, 512], f32r, kind="ExternalInput").ap()
    wp_d = nc.dram_tensor("wp", [512, C], f32r, kind="ExternalInput").ap()
    bqk_d = nc.dram_tensor("bqk", [128, 8], f32, kind="ExternalInput").ap()
    bv_d = nc.dram_tensor("bv", [128, 512], f32, kind="ExternalInput").ap()
    ones_d = nc.dram_tensor("ones64", [128, 64], dt.bfloat16, kind="ExternalInput").ap()
    y_d = nc.dram_tensor("y", [N, C], f32, kind="ExternalOutput").ap()

    with TileContext(nc) as tc:
        with (
            tc.tile_pool(name="persist", bufs=1) as persist,
            tc.tile_pool(name="yqk_pool", bufs=3) as yqk_pool,
            tc.tile_pool(name="es_pool", bufs=22) as es_pool,
            tc.tile_pool(name="norm", bufs=3) as norm,
            tc.tile_pool(name="psum", bufs=2, space="PSUM") as ps,
            tc.tile_pool(name="psum_sm", bufs=2, space="PSUM") as ps_sm,
            tc.tile_pool(name="psav", bufs=2, space="PSUM") as psav,
        ):
            # persistent SBUF tensors
            vst = persist.tile([128, 8, 8, 65], dt.bfloat16, tag="vst")  # [keys128, s, h, d+1]
            attr = [
                [
                    persist.tile(
                        [128, 512], f32r, tag=f"attr{c}_{qc}", name=f"attr{c}_{qc}"
                    )
                    for qc in range(2)
                ]
                for c in range(4)
            ]  # att^T normalized, per (2-head chunk, query half)
            bqk_t = persist.tile([128, 8], f32, tag="bqk")
            bv_t = persist.tile([128, 512], f32, tag="bv")

            nc.gpsimd.dma_start(bqk_t[:], bqk_d)
            nc.gpsimd.dma_start(bv_t[:], bv_d)

            with tc.tile_pool(name="ph1", bufs=1) as ph1:
                xT = [
                    ph1.tile([128, N], f32r, tag=f"xT{k}", name=f"xT{k}")
                    for k in range(8)
                ]
                # wqk column strips [partition, ko, col]; host pre-tiled
                wqs = [
                    ph1.tile([128, 8, 128], f32r, tag=f"wq{cc}", name=f"wq{cc}")
                    for cc in range(8)
                ]
                wv = [
                    ph1.tile([128, 512], f32r, tag=f"wv{k}", name=f"wv{k}")
                    for k in range(8)
                ]
                # ones column of V-hat
                nc.vector.memset(vst[:, :, :, 64], 1.0)
                # DMA priority: pair-0 attention inputs, then V, then the rest
                nc.sync.dma_start(wqs[0][:, 0:4, :], wqk_d[0, :, 0:4, :])
                nc.sync.dma_start(wqs[0][:, 4:8, :], wqk_d[0, :, 4:8, :])
                for k in range(4):
                    nc.sync.dma_start(xT[k][:], xT_d[k * 128:(k + 1) * 128, :])
                nc.sync.dma_start(wqs[4][:], wqk_d[4])
                for k in range(4, 8):
                    nc.sync.dma_start(xT[k][:], xT_d[k * 128:(k + 1) * 128, :])
                nc.sync.dma_start(wqs[1][:], wqk_d[1])
                nc.sync.dma_start(wqs[5][:], wqk_d[5])
                for k in range(8):
                    nc.sync.dma_start(wv[k][:], wv_d[k * 128:(k + 1) * 128, :])
                for p_ in range(2, 4):
                    nc.sync.dma_start(wqs[p_][:], wqk_d[p_])
                    nc.sync.dma_start(wqs[4 + p_][:], wqk_d[4 + p_])

                # ---- per head-pair software pipeline ----
                # Interleave: S^T+exp(p) runs on ACT while PE fills the gaps
                # with AV(p-1) groups (or Y_v for p=0) and the next pair's Yqk.
                yqs, yks = {}, {}

                def emit_yqk(p):
                    for cc, tagn in ((p, "yq"), (4 + p, "yk")):
                        yt = yqk_pool.tile(
                            [128, N], f32r, tag=tagn, name=f"{tagn}{p}"
                        )
                        for s in range(2):
                            pq = ps_sm.tile(
                                [128, 512], f32, tag="sm", name=f"pq{cc}_{s}"
                            )
                            for k in range(8):
                                nc.tensor.matmul(
                                    pq[:],
                                    wqs[cc][:, k, :],
                                    xT[k][:, s * 512:(s + 1) * 512],
                                    start=(k == 0),
                                    stop=(k == 7),
                                )
                            nc.vector.tensor_scalar_add(
                                yt[:, s * 512:(s + 1) * 512],
                                pq[:],
                                bqk_t[:, cc:cc + 1],
                            )
                        (yqs if tagn == "yq" else yks)[p] = yt

                def emit_yv_group(s):
                    pv = ps_sm.tile([128, 512], f32, tag="sm", name=f"pv{s}")
                    for k in range(8):
                        nc.tensor.matmul(
                            pv[:],
                            xT[k][:, s * 128:(s + 1) * 128],
                            wv[k][:],
                            start=(k == 0),
                            stop=(k == 7),
                        )
                    nc.vector.tensor_add(
                        out=vst[:, s, :, 0:64],
                        in0=pv[:].rearrange("p (h d) -> p h d", h=8),
                        in1=bv_t[:].rearrange("p (h d) -> p h d", h=8),
                    )

                def emit_av_group(p, es, j, qc):
                    p0 = j * 64
                    h = 2 * p + j
                    pav = psav.tile([65, 512], f32, tag="av", name=f"pav{h}_{qc}")
                    for kc in range(8):
                        nc.tensor.matmul(
                            pav[:],
                            vst[:, kc, h, :],
                            es[(j, kc)][:, qc * 512:(qc + 1) * 512],
                            start=(kc == 0),
                            stop=(kc == 7),
                        )
                    rc = norm.tile([1, 512], f32, tag="rc", name=f"rc{h}{qc}")
                    nc.vector.reciprocal(rc[:], pav[64:65, :])
                    bc = norm.tile([64, 512], f32, tag="bc", name=f"bc{h}{qc}")
                    nc.gpsimd.partition_broadcast(bc[:], rc[0:1, :])
                    nc.vector.tensor_mul(
                        out=attr[p][qc][p0:p0 + 64, :],
                        in0=pav[0:64, :],
                        in1=bc[:],
                    )

                def emit_st_exp(p, filler):
                    """S^T+exp for pair p; `filler(kc)` emits PE work to
                    overlap the ACT-paced exp stream."""
                    yq, yk = yqs[p], yks[p]
                    es = {}
                    for kc in range(8):
                        psj = [
                            ps.tile([128, N], f32, tag="s", name=f"ps{p}_{j}_{kc}")
                            for j in range(2)
                        ]
                        for qc in range(2):
                            for j, p0 in ((0, 0), (1, 64)):
                                nc.tensor.matmul(
                                    psj[j][:, qc * 512:(qc + 1) * 512],
                                    yk[p0:p0 + 64, kc * 128:(kc + 1) * 128],
                                    yq[p0:p0 + 64, qc * 512:(qc + 1) * 512],
                                    start=True,
                                    stop=True,
                                )
                        for j in range(2):
                            e = es_pool.tile(
                                [128, N], dt.bfloat16, tag="es",
                                name=f"es{p}_{j}_{kc}",
                            )
                            nc.scalar.activation(
                                e[:], psj[j][:], Act.Exp, scale=SCALE
                            )
                            es[(j, kc)] = e
                        filler(kc)
                    return es

                emit_yqk(0)

                def filler0(kc):
                    if kc == 0:
                        emit_yqk(1)
                    if kc >= 2:
                        emit_yv_group(kc - 2)

                es_prev = emit_st_exp(0, filler0)
                emit_yv_group(6)
                emit_yv_group(7)
                prev_p = 0
                for p in range(1, 4):
                    avq = [(j, qc) for j in range(2) for qc in range(2)]

                    def filler(kc, _avq=avq, _pp=prev_p, _es=es_prev, _p=p):
                        if kc % 2 == 1 and _avq:
                            j, qc = _avq.pop(0)
                            emit_av_group(_pp, _es, j, qc)
                        if kc == 4 and _p + 1 < 4:
                            emit_yqk(_p + 1)

                    es_now = emit_st_exp(p, filler)
                    for j, qc in avq:
                        emit_av_group(prev_p, es_prev, j, qc)
                    es_prev, prev_p = es_now, p
            # ---- output projection ----
            with tc.tile_pool(name="proj", bufs=1) as proj:
                wp = [
                    proj.tile([128, 1024], f32r, tag=f"wp{c}", name=f"wp{c}")
                    for c in range(4)
                ]
                for c in range(4):
                    nc.sync.dma_start(wp[c][:], wp_d[c * 128:(c + 1) * 128, :])
                with tc.tile_pool(name="yo_pool", bufs=3) as yo_pool:

                    def emit_proj(st):
                        yo = yo_pool.tile([128, N], f32, tag="yo", name=f"yo{st}")
                        for oc in range(2):
                            pool_sel = ps_sm if (st + oc) % 2 == 0 else ps
                            po = pool_sel.tile(
                                [128, 512], f32,
                                tag="sm" if (st + oc) % 2 == 0 else "s",
                                name=f"po{st}_{oc}",
                            )
                            for c in range(4):
                                nc.tensor.matmul(
                                    po[:],
                                    attr[c][st // 4][
                                        :, (st % 4) * 128:(st % 4 + 1) * 128
                                    ],
                                    wp[c][:, oc * 512:(oc + 1) * 512],
                                    start=(c == 0),
                                    stop=(c == 3),
                                )
                            nc.vector.tensor_copy(
                                yo[:, oc * 512:(oc + 1) * 512], po[:]
                            )
                        nc.sync.dma_start(y_d[st * 128:(st + 1) * 128, :], yo[:])

                    for j in range(2):
                        emit_av_group(3, es_prev, j, 0)
                    for st in range(4):
                        emit_proj(st)
                    for j in range(2):
                        emit_av_group(3, es_prev, j, 1)
                    for st in range(4, 8):
                        emit_proj(st)

    nc.compile()
    return nc


def _get_nc():
    if "nc" not in _NC_CACHE:
        _NC_CACHE["nc"] = _build_bass()
    return _NC_CACHE["nc"]


def _shard_inputs(x, w_qkv, b_qkv, w_proj):
    """Build per-core input maps. core = 2*b + hh."""
    import ml_dtypes

    ones64 = np.ones((128, 64), dtype=ml_dtypes.bfloat16)

    in_maps = []
    for core in range(NCORES):
        b = core // 2
        hh = core % 2
        q_sl = slice(hh * 512, (hh + 1) * 512)
        k_sl = slice(1024 + hh * 512, 1024 + (hh + 1) * 512)
        v_sl = slice(2048 + hh * 512, 2048 + (hh + 1) * 512)

        xT = np.ascontiguousarray(x[b].T)
        wqk = np.concatenate([w_qkv[:, q_sl], w_qkv[:, k_sl]], axis=1)
        # [cc, p, ko, c]: tile (cc) is [partition, ko, col], contiguous per DMA
        wqk = np.ascontiguousarray(
            wqk.reshape(8, 128, 8, 128).transpose(2, 1, 0, 3)
        )
        wv = np.ascontiguousarray(w_qkv[:, v_sl])
        wp = np.ascontiguousarray(w_proj[hh * 512:(hh + 1) * 512, :])
        bqk = np.ascontiguousarray(
            np.concatenate([b_qkv[q_sl], b_qkv[k_sl]]).reshape(8, 128).T
        )
        bv = np.ascontiguousarray(np.broadcast_to(b_qkv[v_sl], (128, 512)))
        in_maps.append(
            {
                "xT": xT,
                "wqk": wqk,
                "wv": wv,
                "wp": wp,
                "bqk": bqk,
                "bv": bv,
                "ones64": ones64,
            }
        )
    return in_maps


def kernel(x, w_qkv, b_qkv, w_proj, b_proj):
    from concourse.bass_utils import run_bass_kernel_spmd

    x = np.asarray(x, dtype=np.float32)
    w_qkv = np.asarray(w_qkv, dtype=np.float32)
    b_qkv = np.asarray(b_qkv, dtype=np.float32)
    w_proj = np.asarray(w_proj, dtype=np.float32)
    b_proj = np.asarray(b_proj, dtype=np.float32)

    nc = _get_nc()
    in_maps = _shard_inputs(x, w_qkv, b_qkv, w_proj)
    res = run_bass_kernel_spmd(nc, in_maps, core_ids=list(range(NCORES)))

    out = np.empty((B, N, C), dtype=np.float32)
    for b in range(B):
        out[b] = res.results[2 * b]["y"] + res.results[2 * b + 1]["y"]
    out += b_proj
    return out
